# revision 10
# baseline (speedup 1.0000x reference)
"""Trainium2 Bass kernel for nn_CrossAttentionLayer (2-stream cross-attention + LN).

Sharding: 8 cores = (stream s) x (batch b) x (query chunk c). Each core handles
1024 query tokens of one (stream, batch): Q projection for its tokens, K/V
projection for the other stream's full 2048 tokens, 16-head cross attention,
out-projection, residual and LayerNorm; host assembles the full output.

Fast path: all matmuls in fp8(e4m3) with fp32 PSUM. Projections and the
attn@V / out-proj matmuls use DoubleRow perf mode (256-deep contraction per
instruction); Q@K scores (contraction=64) use 2x2 PE-array tiling
(tile_position) so 4 matmuls execute concurrently. Softmax exp is split
across the Activation engine (exp table -> fp8) and the Vector engine
(Schraudolph bitcast: int8(x*a+b) reinterpreted as fp8). Softmax denominators
use an int32 reciprocal bit-trick + DMA broadcast. LayerNorm tail runs in
fp16 on the DVE 2x path; output returned fp16 and upcast on host.
"""

import os
import sys

import numpy as np

for _p in ("/opt/trn_rl_repo", "/root/.axon_site/_ro/trn_rl_repo"):
    if os.path.isdir(_p) and _p not in sys.path:
        sys.path.insert(0, _p)

import ml_dtypes

import concourse.bass as bass
import concourse.mybir as mybir
import concourse.tile as tile
from concourse.bass_utils import run_bass_kernel_spmd

F32 = mybir.dt.float32
F16 = mybir.dt.float16
F8 = mybir.dt.float8e4
I32 = mybir.dt.int32
I8 = mybir.dt.int8
ADD = mybir.AluOpType.add
MULT = mybir.AluOpType.mult
EXP = mybir.ActivationFunctionType.Exp
SQRT = mybir.ActivationFunctionType.Sqrt
IDENT = mybir.ActivationFunctionType.Identity
COPY = mybir.ActivationFunctionType.Copy
DR = mybir.MatmulPerfMode.DoubleRow
NP8 = ml_dtypes.float8_e4m3

DIM = 1024
N_TOK = 2048
HEADS = 16
HD = DIM // HEADS        # 64
NQ = 1024                # query tokens per core
S = 2048                 # kv sequence length
P = 128
SCALE = HD ** -0.5
EPS = 1e-5

S_X = 16.0               # embedding fp8 scale
S_W = 128.0              # weight fp8 scale
S_QK = 16.0              # q/k fp8 scale
S_V = 16.0               # v fp8 scale
S_A = 128.0              # attn-out fp8 scale
C_QK = S_QK / (S_X * S_W)
C_V = S_V / (S_X * S_W)
C_E = 1.0 / (S_A * S_W)
M_EXP = SCALE / (S_QK * S_QK)
A_DVE = 8.0 * 1.4426950408889634 * M_EXP
B_DVE = 56.0
RECIP_MAGIC = 0x7EF311C3

# exp engine schedule: 1 = ACT (table exp), 0 = DVE (bitcast trick); 4:3 mix
EXP_PATTERN = (1, 0, 1, 0, 1, 1, 0)

_wsplit_ctr = [0]


def _ensure_ntff_hook():
    """Register the axon NTFF profiling hook if the image lacks
    antenv.axon_hooks (mirrors trn_boot._ntff_profile_via_ctypes)."""
    try:
        from antenv.axon_hooks import get_axon_ntff_profile_hook  # noqa: F401
        return
    except ImportError:
        pass
    import contextlib
    import ctypes
    import types

    try:
        import antenv
    except ImportError:
        return
    mod = types.ModuleType("antenv.axon_hooks")
    _h = [None]
    mod.set_axon_ntff_profile_hook = lambda h: _h.__setitem__(0, h)
    mod.get_axon_ntff_profile_hook = lambda: _h[0]
    sys.modules["antenv.axon_hooks"] = mod
    antenv.axon_hooks = mod

    so_path = "/opt/axon/libaxon_pjrt.so"
    if not os.path.exists(so_path):
        return
    try:
        lib = ctypes.CDLL(so_path)
    except OSError:
        return
    if not hasattr(lib, "axon_start_nrt_profile"):
        return
    lib.axon_start_nrt_profile.argtypes = [
        ctypes.POINTER(ctypes.c_int64),
        ctypes.c_size_t,
    ]
    lib.axon_start_nrt_profile.restype = ctypes.c_int64
    lib.axon_stop_nrt_profile.argtypes = [ctypes.c_char_p]
    lib.axon_stop_nrt_profile.restype = ctypes.c_int64

    @contextlib.contextmanager
    def _hook(output_dir, device_ids):
        import jax

        jax.devices()
        if device_ids:
            ids = (ctypes.c_int64 * len(device_ids))(*device_ids)
            rc = lib.axon_start_nrt_profile(ids, len(device_ids))
        else:
            rc = lib.axon_start_nrt_profile(None, 0)
        if rc != 0:
            raise RuntimeError(f"axon_start_nrt_profile rc={rc}")
        try:
            yield
        finally:
            n = lib.axon_stop_nrt_profile(str(output_dir).encode())
            if n <= 0:
                print(f"profile: rc={n}, no ntff written to {output_dir}")

    mod.set_axon_ntff_profile_hook(_hook)


def _patch_upload_artifacts():
    """Artifact upload needs bucket access this container may not have;
    neuter it (only reachable on trace paths)."""
    from concourse import bass_utils as bu

    bu.upload_artifacts = lambda tmpdir: str(tmpdir)


def _split_sync_waits(nc):
    """This container's walrus build rejects >1 sync-wait per instruction.
    Hoist extra waits onto same-engine NOPs placed just before the instruction
    (engines execute their stream in order, so semantics are preserved)."""
    for f in nc.m.functions:
        for bb in f.blocks:
            insts = bb.instructions
            out = []
            changed = False
            for inst in insts:
                si = inst.sync_info
                if si is not None and si.on_wait and len(si.on_wait) > 1:
                    waits = list(si.on_wait)
                    for w in waits[:-1]:
                        _wsplit_ctr[0] += 1
                        out.append(
                            mybir.InstNoOp(
                                name=f"I-wsplit-{_wsplit_ctr[0]}",
                                engine=inst.engine,
                                ins=[],
                                outs=[],
                                sync_info=mybir.SyncInfo(on_wait=[w], on_update=[]),
                            )
                        )
                    si.on_wait = waits[-1:]
                    changed = True
                out.append(inst)
            if changed:
                insts[:] = out


def _build_bass():
    nc = bass.Bass()
    xT8d = nc.declare_dram_parameter("xT8", [512, 2, NQ], F8, isOutput=False)
    xoT8d = nc.declare_dram_parameter("xoT8", [512, 2, S], F8, isOutput=False)
    wq8d = nc.declare_dram_parameter("wq8", [512, 2, DIM], F8, isOutput=False)
    wk8d = nc.declare_dram_parameter("wk8", [512, 2, DIM], F8, isOutput=False)
    wv8d = nc.declare_dram_parameter("wv8", [512, 2, DIM], F8, isOutput=False)
    wo8d = nc.declare_dram_parameter("wo8", [512, 2, DIM], F8, isOutput=False)
    x_own = nc.declare_dram_parameter("x_own", [NQ, DIM], F32, isOutput=False)
    bqsd = nc.declare_dram_parameter("bqs", [2 * DIM], F32, isOutput=False)
    bvsd = nc.declare_dram_parameter("bvs", [1, DIM], F32, isOutput=False)
    gammad = nc.declare_dram_parameter("gamma16", [1, DIM], F16, isOutput=False)
    betad = nc.declare_dram_parameter("beta16", [1, DIM], F16, isOutput=False)
    y_ext = nc.declare_dram_parameter("y", [NQ, DIM], F16, isOutput=True)

    exp_cnt = [0]
    ev_cnt = [0]

    with tile.TileContext(nc, pool_alloc_mode="queue") as tc:
        from contextlib import ExitStack

        with ExitStack() as ctx:
            const = ctx.enter_context(tc.tile_pool(name="const", bufs=1))
            persist = ctx.enter_context(tc.tile_pool(name="persist", bufs=1))
            dram = ctx.enter_context(tc.tile_pool(name="dram", bufs=8, space="DRAM"))

            # ---- constants ----
            bq_cols = const.tile([P, 16], F32)   # q/k bias feat-major (pre-scaled)
            nc.gpsimd.dma_start(out=bq_cols[:], in_=bqsd[:].rearrange("(t p) -> p t", p=P))
            bv_rep = const.tile([P, DIM], F32)   # v bias broadcast (pre-scaled)
            nc.gpsimd.dma_start(out=bv_rep[:], in_=bvsd[:].to_broadcast([P, DIM]))
            gamma_rep = const.tile([P, DIM], F16)
            nc.gpsimd.dma_start(out=gamma_rep[:], in_=gammad[:].to_broadcast([P, DIM]))
            beta_rep = const.tile([P, DIM], F16)
            nc.gpsimd.dma_start(out=beta_rep[:], in_=betad[:].to_broadcast([P, DIM]))
            eps_t = const.tile([P, 1], F32)
            nc.vector.memset(eps_t[:], EPS)

            # ---- persistent tiles ----
            qT8 = [persist.tile([P, NQ], F8, name=f"qT{f}") for f in range(8)]
            kT8 = [persist.tile([P, S], F8, name=f"kT{f}") for f in range(8)]
            vS8 = [persist.tile([P, 2, HEADS, HD + 1], F8, name=f"vS{kp}")
                   for kp in range(8)]
            aT8 = persist.tile([P, 8, NQ], F8, name="aT8")
            wo8 = [persist.tile([P, 2, DIM], F8, name=f"wo{j}") for j in range(4)]

            def dr_proj(ps, w_tiles, x_tiles, f_lo, f_hi, x_lo, x_hi):
                for j in range(4):
                    nc.tensor.matmul(
                        ps[:],
                        lhsT=w_tiles[j][:, :, f_lo:f_hi],
                        rhs=x_tiles[j][:, :, x_lo:x_hi],
                        start=(j == 0),
                        stop=(j == 3),
                        perf_mode=DR,
                    )

            def evict_qk(dst_ap, ps, bias_ap):
                """psum*C_QK + bias -> fp8, alternating ACT/DVE."""
                ev_cnt[0] += 1
                if ev_cnt[0] % 2 == 0:
                    nc.scalar.activation(dst_ap, ps[:], IDENT, bias=bias_ap,
                                         scale=C_QK)
                else:
                    nc.vector.tensor_scalar(out=dst_ap, in0=ps[:], scalar1=C_QK,
                                            scalar2=bias_ap, op0=MULT, op1=ADD)

            # ======== Phase A: Q projection (DoubleRow) ========
            with (
                tc.tile_pool(name="xq8", bufs=1) as xq8p,
                tc.tile_pool(name="wq8p", bufs=1) as wqp,
                tc.tile_pool(name="psA", bufs=4, space="PSUM") as psA,
            ):
                wq8 = []
                xT8 = []
                for j in range(4):
                    eng = nc.sync if j % 2 == 0 else nc.gpsimd
                    w = wqp.tile([P, 2, DIM], F8, name=f"wq{j}")
                    eng.dma_start(out=w[:], in_=wq8d[j * P:(j + 1) * P])
                    wq8.append(w)
                    x = xq8p.tile([P, 2, NQ], F8, name=f"xT{j}")
                    eng.dma_start(out=x[:], in_=xT8d[j * P:(j + 1) * P])
                    xT8.append(x)
                # queue phase-C loads right behind on the same queue
                xoT8 = []
                wk8 = []
                wv8 = []
                for j in range(4):
                    eng = nc.sync if j % 2 == 0 else nc.gpsimd
                    w = persist.tile([P, 2, DIM], F8, name=f"wk{j}")
                    eng.dma_start(out=w[:], in_=wk8d[j * P:(j + 1) * P])
                    wk8.append(w)
                    x = persist.tile([P, 2, S], F8, name=f"xo{j}")
                    eng.dma_start(out=x[:], in_=xoT8d[j * P:(j + 1) * P])
                    xoT8.append(x)
                for j in range(4):
                    eng = nc.sync if j % 2 == 0 else nc.gpsimd
                    w = persist.tile([P, 2, DIM], F8, name=f"wv{j}")
                    eng.dma_start(out=w[:], in_=wv8d[j * P:(j + 1) * P])
                    w2 = wo8[j]
                    eng.dma_start(out=w2[:], in_=wo8d[j * P:(j + 1) * P])
                    wv8.append(w)

                for f in range(8):
                    for qc in range(2):
                        ps = psA.tile([P, 512], F32, tag="ps")
                        dr_proj(ps, wq8, xT8, f * P, (f + 1) * P,
                                qc * 512, (qc + 1) * 512)
                        evict_qk(qT8[f][:, qc * 512:(qc + 1) * 512], ps,
                                 bq_cols[:, f:f + 1])

            # ======== Phase C: K,V projection (DoubleRow) ========
            with tc.tile_pool(name="psC", bufs=4, space="PSUM") as psC:
                for f in range(8):
                    for kc in range(4):
                        ps = psC.tile([P, 512], F32, tag="ps")
                        dr_proj(ps, wk8, xoT8, f * P, (f + 1) * P,
                                kc * 512, (kc + 1) * 512)
                        evict_qk(kT8[f][:, kc * 512:(kc + 1) * 512], ps,
                                 bq_cols[:, 8 + f:9 + f])
                for kp in range(8):
                    nc.gpsimd.memset(vS8[kp][:, :, :, HD:HD + 1], S_V)
                    for dd in range(2):
                        kt = 2 * kp + dd
                        for half in range(2):
                            ps = psC.tile([P, 512], F32, tag="ps")
                            for j in range(4):
                                nc.tensor.matmul(
                                    ps[:],
                                    lhsT=xoT8[j][:, :, kt * P:(kt + 1) * P],
                                    rhs=wv8[j][:, :, half * 512:(half + 1) * 512],
                                    start=(j == 0),
                                    stop=(j == 3),
                                    perf_mode=DR,
                                )
                            nc.vector.scalar_tensor_tensor(
                                out=vS8[kp][:, dd, half * 8:(half + 1) * 8, 0:HD],
                                in0=ps[:].rearrange("p (h j) -> p h j", j=HD),
                                scalar=C_V,
                                in1=bv_rep[:, half * 512:(half + 1) * 512].rearrange(
                                    "p (h j) -> p h j", j=HD),
                                op0=MULT,
                                op1=ADD,
                            )

            # ======== Phase D: attention + out-proj/LN overlap ========
            with (
                tc.tile_pool(name="psD", bufs=2, space="PSUM") as psD,
                tc.tile_pool(name="pts", bufs=3) as pts,
                tc.tile_pool(name="rrp", bufs=4) as rrp,
                tc.tile_pool(name="stg", bufs=2) as stg,
                tc.tile_pool(name="x32p", bufs=5) as x32p,
                tc.tile_pool(name="ysbp", bufs=5) as ysbp,
                tc.tile_pool(name="lnp", bufs=4) as lnp,
                tc.tile_pool(name="lnq", bufs=2) as lnq,
            ):
                x32s = {}

                def prefetch_x(t):
                    x32 = x32p.tile([P, DIM], F32, tag="x32")
                    nc.gpsimd.dma_start(out=x32[:], in_=x_own[t * P:(t + 1) * P, :])
                    x32s[t] = x32

                y_sbs = {}
                mvs = {}

                def e_mm_block(t):
                    """out-proj matmuls + residual evict + stats for token tile t."""
                    x32 = x32s.pop(t)
                    y_sb = ysbp.tile([P, DIM], F32, tag="ysb")
                    for half in range(2):
                        pe = psD.tile([P, 512], F32, tag="ps", bufs=3)
                        for j in range(4):
                            nc.tensor.matmul(
                                pe[:],
                                lhsT=aT8[:, 2 * j:2 * j + 2, t * P:(t + 1) * P],
                                rhs=wo8[j][:, :, half * 512:(half + 1) * 512],
                                start=(j == 0),
                                stop=(j == 3),
                                perf_mode=DR,
                            )
                        nc.vector.scalar_tensor_tensor(
                            out=y_sb[:, half * 512:(half + 1) * 512],
                            in0=pe[:], scalar=C_E,
                            in1=x32[:, half * 512:(half + 1) * 512],
                            op0=MULT, op1=ADD,
                        )
                    st = lnp.tile([P, 2, 6], F32, tag="bn")
                    nc.vector.bn_stats(st[:, 0, :], y_sb[:, 0:512])
                    nc.vector.bn_stats(st[:, 1, :], y_sb[:, 512:DIM])
                    mv = lnp.tile([P, 2], F32, tag="mv")
                    nc.vector.bn_aggr(mv[:], st[:])
                    y_sbs[t] = y_sb
                    mvs[t] = mv

                def ln_apply_block(qh):
                    """Batched rstd + LN apply + store for token tiles of half qh."""
                    ts = [qh * 4 + i for i in range(4)]
                    var_all = lnq.tile([P, 4], F32, tag="var")
                    nm_all = lnq.tile([P, 4], F32, tag="nm")
                    for i, t in enumerate(ts):
                        mv = mvs.pop(t)
                        nc.vector.tensor_copy(var_all[:, i:i + 1], mv[:, 1:2])
                        nc.vector.tensor_scalar_mul(nm_all[:, i:i + 1],
                                                    mv[:, 0:1], -1.0)
                    sd = lnq.tile([P, 4], F32, tag="sd")
                    nc.scalar.activation(sd[:], var_all[:], SQRT, bias=eps_t[:],
                                         scale=1.0)
                    rstd = lnq.tile([P, 4], F32, tag="rstd")
                    nc.vector.reciprocal(rstd[:], sd[:])
                    for i, t in enumerate(ts):
                        y_sb = y_sbs.pop(t)
                        yn = lnp.tile([P, DIM], F16, tag="yn", bufs=2)
                        nc.vector.tensor_scalar(
                            out=yn[:], in0=y_sb[:], scalar1=nm_all[:, i:i + 1],
                            scalar2=rstd[:, i:i + 1], op0=ADD, op1=MULT,
                        )
                        yg = lnp.tile([P, DIM], F16, tag="yg", bufs=2)
                        nc.vector.tensor_mul(yg[:], yn[:], gamma_rep[:])
                        yo = lnp.tile([P, DIM], F16, tag="yo", bufs=2)
                        nc.vector.tensor_add(yo[:], yg[:], beta_rep[:])
                        nc.gpsimd.dma_start(out=y_ext[t * P:(t + 1) * P, :], in_=yo[:])

                for t in range(4):
                    prefetch_x(t)

                # Deferred-emission FIFO: attnV/normalize/E work trails the
                # scores quads by ~2 iterations so exp latency never stalls
                # the in-order PE queue (keeps HAM at full clock).
                fifo = []
                slot = [0]
                delayed = []

                def pump(target):
                    while len(fifo) > target:
                        fifo.pop(0)()

                def pump_delayed():
                    for item in list(delayed):
                        if item[0] <= slot[0]:
                            delayed.remove(item)
                            item[1]()

                for q in range(2):
                    qsl = slice(q * 512, (q + 1) * 512)
                    if q == 1:
                        for t in range(4, 8):
                            prefetch_x(t)
                    for g in range(8):
                        ctxg = {}

                        def alloc_acc(ctxg=ctxg, q=q, g=g):
                            if "psa" not in ctxg:
                                ctxg["psa"] = psD.tile(
                                    [P, 512], F32, tag="psa", bufs=1,
                                    name=f"psa{q}_{g}")
                                ctxg["psb"] = psD.tile(
                                    [P, 512], F32, tag="psb", bufs=1,
                                    name=f"psb{q}_{g}")
                            return ctxg["psa"], ctxg["psb"]

                        for kp in range(8):
                            pa = psD.tile([P, 1024], F32, tag="ps", bufs=3,
                                          name=f"pa{q}_{g}_{kp}")
                            pb = psD.tile([P, 1024], F32, tag="ps", bufs=3,
                                          name=f"pb{q}_{g}_{kp}")
                            for dd in range(2):
                                kt = 2 * kp + dd
                                for hi in range(2):
                                    tgt = pa if hi == 0 else pb
                                    for c in range(2):
                                        nc.tensor.matmul(
                                            tgt[64 * c:64 * (c + 1),
                                                dd * 512:(dd + 1) * 512],
                                            lhsT=kT8[g][64 * hi:64 * hi + 64,
                                                        kt * P + 64 * c:
                                                        kt * P + 64 * c + 64],
                                            rhs=qT8[g][64 * hi:64 * hi + 64, qsl],
                                            start=True, stop=True,
                                            tile_position=(64 * hi, 64 * c),
                                        )
                            pr = []
                            for hi, srct in ((0, pa), (1, pb)):
                                use_act = (hi == 0) or (kp == 3)
                                exp_cnt[0] += 1
                                if use_act:
                                    pt = pts.tile([P, 1024], F8, bufs=5,
                                                  tag=("pta", "ptb")[hi])
                                    nc.scalar.activation(pt[:], srct[:], EXP,
                                                         scale=M_EXP)
                                    pr.append(pt[:])
                                else:
                                    pt = pts.tile([P, 1024], I8, bufs=5,
                                                  tag=("pta", "ptb")[hi])
                                    nc.vector.tensor_scalar(
                                        out=pt[:], in0=srct[:], scalar1=A_DVE,
                                        scalar2=B_DVE, op0=MULT, op1=ADD,
                                    )
                                    pr.append(pt[:].bitcast(F8))

                            def attn_step(kp=kp, g=g, pr=pr, alloc=alloc_acc):
                                psa, psb = alloc()
                                for hi, tgt in ((0, psa), (1, psb)):
                                    nc.tensor.matmul(
                                        tgt[0:HD + 1, :],
                                        lhsT=vS8[kp][:, :, 2 * g + hi, :],
                                        rhs=pr[hi].rearrange(
                                            "p (a b) -> p a b", a=2),
                                        start=(kp == 0), stop=(kp == 7),
                                        perf_mode=DR,
                                    )
                            fifo.append(attn_step)
                            slot[0] += 1
                            pump_delayed()
                            pump(3)

                        def den_g(ctxg=ctxg, alloc=alloc_acc):
                            """Denominator recip bit-trick + DMA broadcast;
                            emitted right after the last attnV step."""
                            if ctxg.get("den_done"):
                                return
                            ctxg["den_done"] = True
                            psa, psb = alloc()
                            rreps = []
                            for hi, srct in ((0, psa), (1, psb)):
                                den_sb = rrp.tile([1, 512], F32, tag="den")
                                nc.scalar.activation(den_sb[:],
                                                     srct[HD:HD + 1, :], COPY)
                                rri = rrp.tile([1, 512], I32, tag="rri")
                                nc.vector.tensor_scalar(
                                    out=rri[:], in0=den_sb[:].bitcast(I32),
                                    scalar1=-1, scalar2=RECIP_MAGIC,
                                    op0=MULT, op1=ADD,
                                )
                                rdend = dram.tile([1, 512], F32, tag="rden")
                                nc.gpsimd.dma_start(out=rdend[:],
                                                    in_=rri[:].bitcast(F32))
                                rrep = rrp.tile([HD, 512], F32, tag="rrep")
                                nc.gpsimd.dma_start(
                                    out=rrep[:],
                                    in_=rdend[:].to_broadcast([HD, 512]))
                                rreps.append(rrep)
                            ctxg["rreps"] = rreps

                        def norm_g(g=g, qsl=qsl, ctxg=ctxg, alloc=alloc_acc,
                                   den=den_g):
                            """Normalize by broadcast reciprocal -> aT8; popped
                            a few slots after den_g so the DVE never waits on
                            the DMA roundtrip."""
                            if not ctxg.get("den_done"):
                                den(ctxg=ctxg)
                            psa, psb = alloc()
                            rreps = ctxg["rreps"]
                            nc.vector.scalar_tensor_tensor(
                                out=aT8[0:HD, g, qsl], in0=psa[0:HD, :],
                                scalar=S_A, in1=rreps[0][:], op0=MULT, op1=MULT,
                            )
                            st8 = stg.tile([HD, 512], F8, tag="st8")
                            nc.vector.scalar_tensor_tensor(
                                out=st8[:], in0=psb[0:HD, :], scalar=S_A,
                                in1=rreps[1][:], op0=MULT, op1=MULT,
                            )
                            nc.sync.dma_start(out=aT8[HD:P, g, qsl], in_=st8[:])
                        fifo.append(den_g)
                        delayed.append((slot[0] + 3, norm_g))
                        if q == 1 and g % 2 == 1:
                            fifo.append(lambda t=(g - 1) // 2: e_mm_block(t))
                    if q == 1:
                        pump(0)
                        for _, fn in sorted(delayed):
                            fn()
                        delayed.clear()
                        ln_apply_block(0)
                pump(0)
                for _, fn in sorted(delayed):
                    fn()
                delayed.clear()
                for t in range(4, 8):
                    e_mm_block(t)
                ln_apply_block(1)

    _split_sync_waits(nc)
    return nc


_NC_CACHE = None
LAST_RESULT = None


def _get_nc():
    global _NC_CACHE
    if _NC_CACHE is None:
        _NC_CACHE = _build_bass()
    return _NC_CACHE


def _pack_dr_rows(w, scale):
    """[DIM, O] -> [512, 2, O] fp8: row d=256j+128i+p -> [128j+p, i]."""
    O = w.shape[1]
    return np.ascontiguousarray(
        (w * scale).reshape(4, 2, P, O).transpose(0, 2, 1, 3).reshape(512, 2, O)
    ).astype(NP8)


def kernel(embedding1, embedding2, Wqkv, bqkv, Wout, bout, gamma, beta):
    global LAST_RESULT
    embs = [np.asarray(embedding1, dtype=np.float32),
            np.asarray(embedding2, dtype=np.float32)]
    Wqkv = np.asarray(Wqkv, dtype=np.float32)
    Wout = np.asarray(Wout, dtype=np.float32)
    bqkv = np.asarray(bqkv, dtype=np.float32).reshape(3 * DIM)
    bout = np.asarray(bout, dtype=np.float32).reshape(1, DIM)
    gamma = np.asarray(gamma, dtype=np.float32).reshape(1, DIM)
    beta = np.asarray(beta, dtype=np.float32).reshape(1, DIM)

    wq8 = _pack_dr_rows(Wqkv[:, 0:DIM], S_W)
    wk8 = _pack_dr_rows(Wqkv[:, DIM:2 * DIM], S_W)
    wv8 = _pack_dr_rows(Wqkv[:, 2 * DIM:3 * DIM], S_W)
    wo8 = _pack_dr_rows(Wout, S_W)
    bqs = np.ascontiguousarray(bqkv[0:2 * DIM] * S_QK)
    bvs = np.ascontiguousarray(bqkv[2 * DIM:3 * DIM].reshape(1, DIM) * S_V)
    g16 = np.ascontiguousarray(gamma.astype(np.float16))
    b16 = np.ascontiguousarray(beta.astype(np.float16))

    # xT8 per (stream, batch): [DIM, N] fp8 in DR row packing over dims
    xT8 = [[_pack_dr_rows(embs[s][b].T, S_X) for b in range(2)] for s in range(2)]

    nc = _get_nc()
    in_maps = []
    layout = []
    for s in range(2):
        for b in range(2):
            for c in range(2):
                in_maps.append({
                    "xT8": np.ascontiguousarray(xT8[s][b][:, :, c * NQ:(c + 1) * NQ]),
                    "xoT8": xT8[1 - s][b],
                    "wq8": wq8, "wk8": wk8, "wv8": wv8, "wo8": wo8,
                    "x_own": np.ascontiguousarray(
                        embs[s][b, c * NQ:(c + 1) * NQ, :] + bout),
                    "bqs": bqs, "bvs": bvs,
                    "gamma16": g16, "beta16": b16,
                })
                layout.append((s, b, c))

    trace = os.environ.get("TRN_KERNEL_TRACE", "") not in ("", "0")
    if trace:
        _ensure_ntff_hook()
        _patch_upload_artifacts()
    res = run_bass_kernel_spmd(
        nc, in_maps, core_ids=list(range(8)), trace=trace,
    )
    LAST_RESULT = res

    out = np.zeros((2, 2, N_TOK, DIM), dtype=np.float32)
    for i, (s, b, c) in enumerate(layout):
        out[s, b, c * NQ:(c + 1) * NQ, :] = np.asarray(
            res.results[i]["y"]).astype(np.float32)
    return out


# revision 11
# speedup vs baseline: 1.0249x; 1.0249x over previous
"""Trainium2 Bass kernel for nn_CrossAttentionLayer (2-stream cross-attention + LN).

Sharding: 8 cores = (stream s) x (batch b) x (query chunk c). Each core handles
1024 query tokens of one (stream, batch): Q projection for its tokens, K/V
projection for the other stream's full 2048 tokens, 16-head cross attention,
out-projection, residual and LayerNorm; host assembles the full output.

Fast path: all matmuls in fp8(e4m3) with fp32 PSUM. Projections and the
attn@V / out-proj matmuls use DoubleRow perf mode (256-deep contraction per
instruction); Q@K scores (contraction=64) use 2x2 PE-array tiling
(tile_position) so 4 matmuls execute concurrently. Softmax exp is split
across the Activation engine (exp table -> fp8) and the Vector engine
(Schraudolph bitcast: int8(x*a+b) reinterpreted as fp8). Softmax denominators
use an int32 reciprocal bit-trick + DMA broadcast. LayerNorm tail runs in
fp16 on the DVE 2x path; output returned fp16 and upcast on host.
"""

import os
import sys

import numpy as np

for _p in ("/opt/trn_rl_repo", "/root/.axon_site/_ro/trn_rl_repo"):
    if os.path.isdir(_p) and _p not in sys.path:
        sys.path.insert(0, _p)

import ml_dtypes

import concourse.bass as bass
import concourse.mybir as mybir
import concourse.tile as tile
from concourse.bass_utils import run_bass_kernel_spmd

F32 = mybir.dt.float32
F16 = mybir.dt.float16
F8 = mybir.dt.float8e4
I32 = mybir.dt.int32
I8 = mybir.dt.int8
ADD = mybir.AluOpType.add
MULT = mybir.AluOpType.mult
EXP = mybir.ActivationFunctionType.Exp
SQRT = mybir.ActivationFunctionType.Sqrt
IDENT = mybir.ActivationFunctionType.Identity
COPY = mybir.ActivationFunctionType.Copy
DR = mybir.MatmulPerfMode.DoubleRow
NP8 = ml_dtypes.float8_e4m3

DIM = 1024
N_TOK = 2048
HEADS = 16
HD = DIM // HEADS        # 64
NQ = 1024                # query tokens per core
S = 2048                 # kv sequence length
P = 128
SCALE = HD ** -0.5
EPS = 1e-5

S_X = 16.0               # embedding fp8 scale
S_W = 128.0              # weight fp8 scale
S_QK = 16.0              # q/k fp8 scale
S_V = 16.0               # v fp8 scale
S_A = 128.0              # attn-out fp8 scale
C_QK = S_QK / (S_X * S_W)
C_V = S_V / (S_X * S_W)
C_E = 1.0 / (S_A * S_W)
M_EXP = SCALE / (S_QK * S_QK)
A_DVE = 8.0 * 1.4426950408889634 * M_EXP
B_DVE = 56.0
RECIP_MAGIC = 0x7EF311C3

# exp engine schedule: 1 = ACT (table exp), 0 = DVE (bitcast trick); 4:3 mix
EXP_PATTERN = (1, 0, 1, 0, 1, 1, 0)

_wsplit_ctr = [0]


def _ensure_ntff_hook():
    """Register the axon NTFF profiling hook if the image lacks
    antenv.axon_hooks (mirrors trn_boot._ntff_profile_via_ctypes)."""
    try:
        from antenv.axon_hooks import get_axon_ntff_profile_hook  # noqa: F401
        return
    except ImportError:
        pass
    import contextlib
    import ctypes
    import types

    try:
        import antenv
    except ImportError:
        return
    mod = types.ModuleType("antenv.axon_hooks")
    _h = [None]
    mod.set_axon_ntff_profile_hook = lambda h: _h.__setitem__(0, h)
    mod.get_axon_ntff_profile_hook = lambda: _h[0]
    sys.modules["antenv.axon_hooks"] = mod
    antenv.axon_hooks = mod

    so_path = "/opt/axon/libaxon_pjrt.so"
    if not os.path.exists(so_path):
        return
    try:
        lib = ctypes.CDLL(so_path)
    except OSError:
        return
    if not hasattr(lib, "axon_start_nrt_profile"):
        return
    lib.axon_start_nrt_profile.argtypes = [
        ctypes.POINTER(ctypes.c_int64),
        ctypes.c_size_t,
    ]
    lib.axon_start_nrt_profile.restype = ctypes.c_int64
    lib.axon_stop_nrt_profile.argtypes = [ctypes.c_char_p]
    lib.axon_stop_nrt_profile.restype = ctypes.c_int64

    @contextlib.contextmanager
    def _hook(output_dir, device_ids):
        import jax

        jax.devices()
        if device_ids:
            ids = (ctypes.c_int64 * len(device_ids))(*device_ids)
            rc = lib.axon_start_nrt_profile(ids, len(device_ids))
        else:
            rc = lib.axon_start_nrt_profile(None, 0)
        if rc != 0:
            raise RuntimeError(f"axon_start_nrt_profile rc={rc}")
        try:
            yield
        finally:
            n = lib.axon_stop_nrt_profile(str(output_dir).encode())
            if n <= 0:
                print(f"profile: rc={n}, no ntff written to {output_dir}")

    mod.set_axon_ntff_profile_hook(_hook)


def _patch_upload_artifacts():
    """Artifact upload needs bucket access this container may not have;
    neuter it (only reachable on trace paths)."""
    from concourse import bass_utils as bu

    bu.upload_artifacts = lambda tmpdir: str(tmpdir)


def _split_sync_waits(nc):
    """This container's walrus build rejects >1 sync-wait per instruction.
    Hoist extra waits onto same-engine NOPs placed just before the instruction
    (engines execute their stream in order, so semantics are preserved)."""
    for f in nc.m.functions:
        for bb in f.blocks:
            insts = bb.instructions
            out = []
            changed = False
            for inst in insts:
                si = inst.sync_info
                if si is not None and si.on_wait and len(si.on_wait) > 1:
                    waits = list(si.on_wait)
                    for w in waits[:-1]:
                        _wsplit_ctr[0] += 1
                        out.append(
                            mybir.InstNoOp(
                                name=f"I-wsplit-{_wsplit_ctr[0]}",
                                engine=inst.engine,
                                ins=[],
                                outs=[],
                                sync_info=mybir.SyncInfo(on_wait=[w], on_update=[]),
                            )
                        )
                    si.on_wait = waits[-1:]
                    changed = True
                out.append(inst)
            if changed:
                insts[:] = out


def _build_bass():
    nc = bass.Bass()
    xT8d = nc.declare_dram_parameter("xT8", [512, 2, NQ], F8, isOutput=False)
    xoT8d = nc.declare_dram_parameter("xoT8", [512, 2, S], F8, isOutput=False)
    wq8d = nc.declare_dram_parameter("wq8", [512, 2, DIM], F8, isOutput=False)
    wk8d = nc.declare_dram_parameter("wk8", [512, 2, DIM], F8, isOutput=False)
    wv8d = nc.declare_dram_parameter("wv8", [512, 2, DIM], F8, isOutput=False)
    wo8d = nc.declare_dram_parameter("wo8", [512, 2, DIM], F8, isOutput=False)
    x_own = nc.declare_dram_parameter("x_own", [NQ, DIM], F32, isOutput=False)
    bqsd = nc.declare_dram_parameter("bqs", [2 * DIM], F32, isOutput=False)
    bvsd = nc.declare_dram_parameter("bvs", [1, DIM], F32, isOutput=False)
    gammad = nc.declare_dram_parameter("gamma16", [1, DIM], F16, isOutput=False)
    betad = nc.declare_dram_parameter("beta16", [1, DIM], F16, isOutput=False)
    y_ext = nc.declare_dram_parameter("y", [NQ, DIM], F16, isOutput=True)

    exp_cnt = [0]
    ev_cnt = [0]

    with tile.TileContext(nc, pool_alloc_mode="queue") as tc:
        from contextlib import ExitStack

        with ExitStack() as ctx:
            const = ctx.enter_context(tc.tile_pool(name="const", bufs=1))
            persist = ctx.enter_context(tc.tile_pool(name="persist", bufs=1))
            dram = ctx.enter_context(tc.tile_pool(name="dram", bufs=8, space="DRAM"))

            # ---- constants ----
            bq_cols = const.tile([P, 16], F32)   # q/k bias feat-major (pre-scaled)
            nc.scalar.dma_start(out=bq_cols[:], in_=bqsd[:].rearrange("(t p) -> p t", p=P))
            bv_rep = const.tile([P, DIM], F32)   # v bias broadcast (pre-scaled)
            nc.scalar.dma_start(out=bv_rep[:], in_=bvsd[:].to_broadcast([P, DIM]))
            gamma_rep = const.tile([P, DIM], F16)
            nc.scalar.dma_start(out=gamma_rep[:], in_=gammad[:].to_broadcast([P, DIM]))
            beta_rep = const.tile([P, DIM], F16)
            nc.scalar.dma_start(out=beta_rep[:], in_=betad[:].to_broadcast([P, DIM]))
            eps_t = const.tile([P, 1], F32)
            nc.vector.memset(eps_t[:], EPS)

            # ---- persistent tiles ----
            qT8 = [persist.tile([P, NQ], F8, name=f"qT{f}") for f in range(8)]
            kT8 = [persist.tile([P, S], F8, name=f"kT{f}") for f in range(8)]
            vS8 = [persist.tile([P, 2, HEADS, HD + 1], F8, name=f"vS{kp}")
                   for kp in range(8)]
            aT8 = persist.tile([P, 8, NQ], F8, name="aT8")
            wo8 = [persist.tile([P, 2, DIM], F8, name=f"wo{j}") for j in range(4)]

            def dr_proj(ps, w_tiles, x_tiles, f_lo, f_hi, x_lo, x_hi):
                for j in range(4):
                    nc.tensor.matmul(
                        ps[:],
                        lhsT=w_tiles[j][:, :, f_lo:f_hi],
                        rhs=x_tiles[j][:, :, x_lo:x_hi],
                        start=(j == 0),
                        stop=(j == 3),
                        perf_mode=DR,
                    )

            def evict_qk(dst_ap, ps, bias_ap):
                """psum*C_QK + bias -> fp8, alternating ACT/DVE."""
                ev_cnt[0] += 1
                if ev_cnt[0] % 2 == 0:
                    nc.scalar.activation(dst_ap, ps[:], IDENT, bias=bias_ap,
                                         scale=C_QK)
                else:
                    nc.vector.tensor_scalar(out=dst_ap, in0=ps[:], scalar1=C_QK,
                                            scalar2=bias_ap, op0=MULT, op1=ADD)

            # ======== Phase A: Q projection (DoubleRow) ========
            with (
                tc.tile_pool(name="xq8", bufs=1) as xq8p,
                tc.tile_pool(name="wq8p", bufs=1) as wqp,
                tc.tile_pool(name="psA", bufs=4, space="PSUM") as psA,
            ):
                wq8 = []
                xT8 = []
                for j in range(4):
                    eng = nc.sync if j % 2 == 0 else nc.gpsimd
                    w = wqp.tile([P, 2, DIM], F8, name=f"wq{j}")
                    eng.dma_start(out=w[:], in_=wq8d[j * P:(j + 1) * P])
                    wq8.append(w)
                    x = xq8p.tile([P, 2, NQ], F8, name=f"xT{j}")
                    eng.dma_start(out=x[:], in_=xT8d[j * P:(j + 1) * P])
                    xT8.append(x)
                # queue phase-C loads right behind on the same queue
                xoT8 = []
                wk8 = []
                wv8 = []
                for j in range(4):
                    eng = nc.sync if j % 2 == 0 else nc.gpsimd
                    w = persist.tile([P, 2, DIM], F8, name=f"wk{j}")
                    eng.dma_start(out=w[:], in_=wk8d[j * P:(j + 1) * P])
                    wk8.append(w)
                    x = persist.tile([P, 2, S], F8, name=f"xo{j}")
                    eng.dma_start(out=x[:], in_=xoT8d[j * P:(j + 1) * P])
                    xoT8.append(x)
                for j in range(4):
                    eng = nc.sync if j % 2 == 0 else nc.gpsimd
                    w = persist.tile([P, 2, DIM], F8, name=f"wv{j}")
                    eng.dma_start(out=w[:], in_=wv8d[j * P:(j + 1) * P])
                    w2 = wo8[j]
                    eng.dma_start(out=w2[:], in_=wo8d[j * P:(j + 1) * P])
                    wv8.append(w)

                for f in range(8):
                    for qc in range(2):
                        ps = psA.tile([P, 512], F32, tag="ps")
                        dr_proj(ps, wq8, xT8, f * P, (f + 1) * P,
                                qc * 512, (qc + 1) * 512)
                        evict_qk(qT8[f][:, qc * 512:(qc + 1) * 512], ps,
                                 bq_cols[:, f:f + 1])

            # ======== Phase C: K,V projection (DoubleRow) ========
            with tc.tile_pool(name="psC", bufs=4, space="PSUM") as psC:
                for f in range(8):
                    for kc in range(4):
                        ps = psC.tile([P, 512], F32, tag="ps")
                        dr_proj(ps, wk8, xoT8, f * P, (f + 1) * P,
                                kc * 512, (kc + 1) * 512)
                        evict_qk(kT8[f][:, kc * 512:(kc + 1) * 512], ps,
                                 bq_cols[:, 8 + f:9 + f])
                for kp in range(8):
                    nc.gpsimd.memset(vS8[kp][:, :, :, HD:HD + 1], S_V)
                    for dd in range(2):
                        kt = 2 * kp + dd
                        for half in range(2):
                            ps = psC.tile([P, 512], F32, tag="ps")
                            for j in range(4):
                                nc.tensor.matmul(
                                    ps[:],
                                    lhsT=xoT8[j][:, :, kt * P:(kt + 1) * P],
                                    rhs=wv8[j][:, :, half * 512:(half + 1) * 512],
                                    start=(j == 0),
                                    stop=(j == 3),
                                    perf_mode=DR,
                                )
                            nc.vector.scalar_tensor_tensor(
                                out=vS8[kp][:, dd, half * 8:(half + 1) * 8, 0:HD],
                                in0=ps[:].rearrange("p (h j) -> p h j", j=HD),
                                scalar=C_V,
                                in1=bv_rep[:, half * 512:(half + 1) * 512].rearrange(
                                    "p (h j) -> p h j", j=HD),
                                op0=MULT,
                                op1=ADD,
                            )

            # ======== Phase D: attention + out-proj/LN overlap ========
            with (
                tc.tile_pool(name="psD", bufs=2, space="PSUM") as psD,
                tc.tile_pool(name="pts", bufs=3) as pts,
                tc.tile_pool(name="rrp", bufs=4) as rrp,
                tc.tile_pool(name="stg", bufs=2) as stg,
                tc.tile_pool(name="x32p", bufs=5) as x32p,
                tc.tile_pool(name="ysbp", bufs=5) as ysbp,
                tc.tile_pool(name="lnp", bufs=4) as lnp,
                tc.tile_pool(name="lnq", bufs=2) as lnq,
            ):
                x32s = {}

                def prefetch_x(t):
                    x32 = x32p.tile([P, DIM], F32, tag="x32")
                    nc.gpsimd.dma_start(out=x32[:], in_=x_own[t * P:(t + 1) * P, :])
                    x32s[t] = x32

                y_sbs = {}
                mvs = {}

                def e_mm_block(t):
                    """out-proj matmuls + residual evict + stats for token tile t."""
                    x32 = x32s.pop(t)
                    y_sb = ysbp.tile([P, DIM], F32, tag="ysb")
                    for half in range(2):
                        pe = psD.tile([P, 512], F32, tag="ps", bufs=3)
                        for j in range(4):
                            nc.tensor.matmul(
                                pe[:],
                                lhsT=aT8[:, 2 * j:2 * j + 2, t * P:(t + 1) * P],
                                rhs=wo8[j][:, :, half * 512:(half + 1) * 512],
                                start=(j == 0),
                                stop=(j == 3),
                                perf_mode=DR,
                            )
                        nc.vector.scalar_tensor_tensor(
                            out=y_sb[:, half * 512:(half + 1) * 512],
                            in0=pe[:], scalar=C_E,
                            in1=x32[:, half * 512:(half + 1) * 512],
                            op0=MULT, op1=ADD,
                        )
                    st = lnp.tile([P, 2, 6], F32, tag="bn")
                    nc.vector.bn_stats(st[:, 0, :], y_sb[:, 0:512])
                    nc.vector.bn_stats(st[:, 1, :], y_sb[:, 512:DIM])
                    mv = lnp.tile([P, 2], F32, tag="mv")
                    nc.vector.bn_aggr(mv[:], st[:])
                    y_sbs[t] = y_sb
                    mvs[t] = mv

                def ln_apply_block(qh):
                    """Batched rstd + LN apply + store for token tiles of half qh."""
                    ts = [qh * 4 + i for i in range(4)]
                    var_all = lnq.tile([P, 4], F32, tag="var")
                    nm_all = lnq.tile([P, 4], F32, tag="nm")
                    for i, t in enumerate(ts):
                        mv = mvs.pop(t)
                        nc.vector.tensor_copy(var_all[:, i:i + 1], mv[:, 1:2])
                        nc.vector.tensor_scalar_mul(nm_all[:, i:i + 1],
                                                    mv[:, 0:1], -1.0)
                    sd = lnq.tile([P, 4], F32, tag="sd")
                    nc.scalar.activation(sd[:], var_all[:], SQRT, bias=eps_t[:],
                                         scale=1.0)
                    rstd = lnq.tile([P, 4], F32, tag="rstd")
                    nc.vector.reciprocal(rstd[:], sd[:])
                    for i, t in enumerate(ts):
                        y_sb = y_sbs.pop(t)
                        yn = lnp.tile([P, DIM], F16, tag="yn", bufs=2)
                        nc.vector.tensor_scalar(
                            out=yn[:], in0=y_sb[:], scalar1=nm_all[:, i:i + 1],
                            scalar2=rstd[:, i:i + 1], op0=ADD, op1=MULT,
                        )
                        yg = lnp.tile([P, DIM], F16, tag="yg", bufs=2)
                        nc.vector.tensor_mul(yg[:], yn[:], gamma_rep[:])
                        yo = lnp.tile([P, DIM], F16, tag="yo", bufs=2)
                        nc.vector.tensor_add(yo[:], yg[:], beta_rep[:])
                        nc.gpsimd.dma_start(out=y_ext[t * P:(t + 1) * P, :], in_=yo[:])

                for t in range(4):
                    prefetch_x(t)

                # Deferred-emission FIFO: attnV/normalize/E work trails the
                # scores quads by ~2 iterations so exp latency never stalls
                # the in-order PE queue (keeps HAM at full clock).
                fifo = []
                slot = [0]
                delayed = []

                def pump(target):
                    while len(fifo) > target:
                        fifo.pop(0)()

                def pump_delayed():
                    for item in list(delayed):
                        if item[0] <= slot[0]:
                            delayed.remove(item)
                            item[1]()

                for q in range(2):
                    qsl = slice(q * 512, (q + 1) * 512)
                    if q == 1:
                        for t in range(4, 8):
                            prefetch_x(t)
                    for g in range(8):
                        ctxg = {}

                        def alloc_acc(ctxg=ctxg, q=q, g=g):
                            if "psa" not in ctxg:
                                ctxg["psa"] = psD.tile(
                                    [P, 512], F32, tag="psa", bufs=1,
                                    name=f"psa{q}_{g}")
                                ctxg["psb"] = psD.tile(
                                    [P, 512], F32, tag="psb", bufs=1,
                                    name=f"psb{q}_{g}")
                            return ctxg["psa"], ctxg["psb"]

                        for kp in range(8):
                            pa = psD.tile([P, 1024], F32, tag="ps", bufs=3,
                                          name=f"pa{q}_{g}_{kp}")
                            pb = psD.tile([P, 1024], F32, tag="ps", bufs=3,
                                          name=f"pb{q}_{g}_{kp}")
                            for dd in range(2):
                                kt = 2 * kp + dd
                                for hi in range(2):
                                    tgt = pa if hi == 0 else pb
                                    for c in range(2):
                                        nc.tensor.matmul(
                                            tgt[64 * c:64 * (c + 1),
                                                dd * 512:(dd + 1) * 512],
                                            lhsT=kT8[g][64 * hi:64 * hi + 64,
                                                        kt * P + 64 * c:
                                                        kt * P + 64 * c + 64],
                                            rhs=qT8[g][64 * hi:64 * hi + 64, qsl],
                                            start=True, stop=True,
                                            tile_position=(64 * hi, 64 * c),
                                        )
                            pr = []
                            for hi, srct in ((0, pa), (1, pb)):
                                use_act = (hi == 0) or (kp == 3)
                                exp_cnt[0] += 1
                                if use_act:
                                    pt = pts.tile([P, 1024], F8, bufs=5,
                                                  tag=("pta", "ptb")[hi])
                                    nc.scalar.activation(pt[:], srct[:], EXP,
                                                         scale=M_EXP)
                                    pr.append(pt[:])
                                else:
                                    pt = pts.tile([P, 1024], I8, bufs=5,
                                                  tag=("pta", "ptb")[hi])
                                    nc.vector.tensor_scalar(
                                        out=pt[:], in0=srct[:], scalar1=A_DVE,
                                        scalar2=B_DVE, op0=MULT, op1=ADD,
                                    )
                                    pr.append(pt[:].bitcast(F8))

                            def attn_step(kp=kp, g=g, pr=pr, alloc=alloc_acc):
                                psa, psb = alloc()
                                for hi, tgt in ((0, psa), (1, psb)):
                                    nc.tensor.matmul(
                                        tgt[0:HD + 1, :],
                                        lhsT=vS8[kp][:, :, 2 * g + hi, :],
                                        rhs=pr[hi].rearrange(
                                            "p (a b) -> p a b", a=2),
                                        start=(kp == 0), stop=(kp == 7),
                                        perf_mode=DR,
                                    )
                            fifo.append(attn_step)
                            slot[0] += 1
                            pump_delayed()
                            pump(2)

                        def den_g(ctxg=ctxg, alloc=alloc_acc):
                            """Denominator recip bit-trick + DMA broadcast;
                            emitted right after the last attnV step."""
                            if ctxg.get("den_done"):
                                return
                            ctxg["den_done"] = True
                            psa, psb = alloc()
                            rreps = []
                            for hi, srct in ((0, psa), (1, psb)):
                                den_sb = rrp.tile([1, 512], F32, tag="den")
                                nc.scalar.activation(den_sb[:],
                                                     srct[HD:HD + 1, :], COPY)
                                rri = rrp.tile([1, 512], I32, tag="rri")
                                nc.vector.tensor_scalar(
                                    out=rri[:], in0=den_sb[:].bitcast(I32),
                                    scalar1=-1, scalar2=RECIP_MAGIC,
                                    op0=MULT, op1=ADD,
                                )
                                rdend = dram.tile([1, 512], F32, tag="rden")
                                nc.gpsimd.dma_start(out=rdend[:],
                                                    in_=rri[:].bitcast(F32))
                                rrep = rrp.tile([HD, 512], F32, tag="rrep")
                                nc.gpsimd.dma_start(
                                    out=rrep[:],
                                    in_=rdend[:].to_broadcast([HD, 512]))
                                rreps.append(rrep)
                            ctxg["rreps"] = rreps

                        def norm_g(g=g, qsl=qsl, ctxg=ctxg, alloc=alloc_acc,
                                   den=den_g):
                            """Normalize by broadcast reciprocal -> aT8; popped
                            a few slots after den_g so the DVE never waits on
                            the DMA roundtrip."""
                            if not ctxg.get("den_done"):
                                den(ctxg=ctxg)
                            psa, psb = alloc()
                            rreps = ctxg["rreps"]
                            nc.vector.scalar_tensor_tensor(
                                out=aT8[0:HD, g, qsl], in0=psa[0:HD, :],
                                scalar=S_A, in1=rreps[0][:], op0=MULT, op1=MULT,
                            )
                            st8 = stg.tile([HD, 512], F8, tag="st8")
                            nc.vector.scalar_tensor_tensor(
                                out=st8[:], in0=psb[0:HD, :], scalar=S_A,
                                in1=rreps[1][:], op0=MULT, op1=MULT,
                            )
                            nc.sync.dma_start(out=aT8[HD:P, g, qsl], in_=st8[:])
                        fifo.append(den_g)
                        delayed.append((slot[0] + 3, norm_g))
                        if q == 1 and g % 2 == 1:
                            fifo.append(lambda t=(g - 1) // 2: e_mm_block(t))
                    if q == 1:
                        pump(0)
                        for _, fn in sorted(delayed):
                            fn()
                        delayed.clear()
                        ln_apply_block(0)
                pump(0)
                for _, fn in sorted(delayed):
                    fn()
                delayed.clear()
                for t in range(4, 8):
                    e_mm_block(t)
                ln_apply_block(1)

    _split_sync_waits(nc)
    return nc


_NC_CACHE = None
LAST_RESULT = None


def _get_nc():
    global _NC_CACHE
    if _NC_CACHE is None:
        _NC_CACHE = _build_bass()
    return _NC_CACHE


def _pack_dr_rows(w, scale):
    """[DIM, O] -> [512, 2, O] fp8: row d=256j+128i+p -> [128j+p, i]."""
    O = w.shape[1]
    return np.ascontiguousarray(
        (w * scale).reshape(4, 2, P, O).transpose(0, 2, 1, 3).reshape(512, 2, O)
    ).astype(NP8)


def kernel(embedding1, embedding2, Wqkv, bqkv, Wout, bout, gamma, beta):
    global LAST_RESULT
    embs = [np.asarray(embedding1, dtype=np.float32),
            np.asarray(embedding2, dtype=np.float32)]
    Wqkv = np.asarray(Wqkv, dtype=np.float32)
    Wout = np.asarray(Wout, dtype=np.float32)
    bqkv = np.asarray(bqkv, dtype=np.float32).reshape(3 * DIM)
    bout = np.asarray(bout, dtype=np.float32).reshape(1, DIM)
    gamma = np.asarray(gamma, dtype=np.float32).reshape(1, DIM)
    beta = np.asarray(beta, dtype=np.float32).reshape(1, DIM)

    wq8 = _pack_dr_rows(Wqkv[:, 0:DIM], S_W)
    wk8 = _pack_dr_rows(Wqkv[:, DIM:2 * DIM], S_W)
    wv8 = _pack_dr_rows(Wqkv[:, 2 * DIM:3 * DIM], S_W)
    wo8 = _pack_dr_rows(Wout, S_W)
    bqs = np.ascontiguousarray(bqkv[0:2 * DIM] * S_QK)
    bvs = np.ascontiguousarray(bqkv[2 * DIM:3 * DIM].reshape(1, DIM) * S_V)
    g16 = np.ascontiguousarray(gamma.astype(np.float16))
    b16 = np.ascontiguousarray(beta.astype(np.float16))

    # xT8 per (stream, batch): [DIM, N] fp8 in DR row packing over dims
    xT8 = [[_pack_dr_rows(embs[s][b].T, S_X) for b in range(2)] for s in range(2)]

    nc = _get_nc()
    in_maps = []
    layout = []
    for s in range(2):
        for b in range(2):
            for c in range(2):
                in_maps.append({
                    "xT8": np.ascontiguousarray(xT8[s][b][:, :, c * NQ:(c + 1) * NQ]),
                    "xoT8": xT8[1 - s][b],
                    "wq8": wq8, "wk8": wk8, "wv8": wv8, "wo8": wo8,
                    "x_own": np.ascontiguousarray(
                        embs[s][b, c * NQ:(c + 1) * NQ, :] + bout),
                    "bqs": bqs, "bvs": bvs,
                    "gamma16": g16, "beta16": b16,
                })
                layout.append((s, b, c))

    trace = os.environ.get("TRN_KERNEL_TRACE", "") not in ("", "0")
    if trace:
        _ensure_ntff_hook()
        _patch_upload_artifacts()
    res = run_bass_kernel_spmd(
        nc, in_maps, core_ids=list(range(8)), trace=trace,
    )
    LAST_RESULT = res

    out = np.zeros((2, 2, N_TOK, DIM), dtype=np.float32)
    for i, (s, b, c) in enumerate(layout):
        out[s, b, c * NQ:(c + 1) * NQ, :] = np.asarray(
            res.results[i]["y"]).astype(np.float32)
    return out
